# revision 3
# baseline (speedup 1.0000x reference)
"""Trainium2 Bass kernel for nn_CompressK (segment_reduce).

Computes, per sequence, a mean over sliding windows of KERNEL_SIZE=32 rows
at stride KERNEL_STRIDE=16 of k (viewed as (rows, head_num_k*head_dim)),
returning (compressed_k, cu_comp) exactly like the reference.

Hardware strategy (8 NeuronCores):
  - 4 sequences of 16384 rows -> 1023 chunks each. Two cores per sequence:
    core 2s   computes chunks   0..511 (rows [0,     8208) of seq s)
    core 2s+1 computes chunks 511..1022 (rows [8176, 16384) of seq s)
    Both produce 512 chunks; the duplicated chunk 511 is dropped on gather.
  - Per core the windowed mean is a banded matmul: out[m, f] =
    sum_p W[p, m] * rows[p, f], with W a 32-wide band of 1/32.
    4 PSUM groups of 128 chunks; each group = 17 accumulating float32r
    matmuls (16x K=128 over a 4 MB batched DMA tile + 1x K=16 tail).
"""

import numpy as np

KERNEL_SIZE = 32
KERNEL_STRIDE = 16
HEAD_NUM_K = 4
HEAD_DIM = 128
BATCH = 4
SEQ_LEN = 16384
F = HEAD_NUM_K * HEAD_DIM          # 512 features per row
N_CORES = 8
CHUNKS_PER_SEQ = (SEQ_LEN - KERNEL_SIZE) // KERNEL_STRIDE + 1  # 1023
CHUNKS_PER_CORE = 512
GROUPS = 4                         # PSUM groups of 128 chunks per core
GROUP_ROWS = 128 * KERNEL_STRIDE   # 2048 rows per group window start span
ROWS_PER_CORE = GROUPS * GROUP_ROWS + (KERNEL_SIZE - KERNEL_STRIDE)  # 8208
W_TILES = 16                       # full 128-row weight tiles per group

_CACHE = {}


def _build_weights() -> np.ndarray:
    """(17, 128, 128) fp32: wb[t, p, m] = 1/32 iff row 128t+p is in chunk m's
    window [16m, 16m+32). Tile 16 only uses rows 0..15 (tail of the group)."""
    wfull = np.zeros(((W_TILES + 1) * 128, 128), np.float32)
    for m in range(128):
        wfull[KERNEL_STRIDE * m: KERNEL_STRIDE * m + KERNEL_SIZE, m] = 1.0 / KERNEL_SIZE
    return np.ascontiguousarray(wfull.reshape(W_TILES + 1, 128, 128))


def _build_module():
    import concourse.tile as tile
    from concourse import bacc, mybir

    nc = bacc.Bacc("TRN2", target_bir_lowering=False, debug=False)
    kx = nc.dram_tensor("kx", [ROWS_PER_CORE, F], mybir.dt.float32,
                        kind="ExternalInput").ap()
    wb = nc.dram_tensor("wb", [W_TILES + 1, 128, 128], mybir.dt.float32,
                        kind="ExternalInput").ap()
    out = nc.dram_tensor("out", [CHUNKS_PER_CORE, F], mybir.dt.float32,
                         kind="ExternalOutput").ap()
    f32r = mybir.dt.float32r

    with tile.TileContext(nc) as tc:
        with tc.tile_pool(name="wpool", bufs=1) as wpool, \
             tc.tile_pool(name="data", bufs=3) as dpool, \
             tc.tile_pool(name="tail", bufs=2) as tpool, \
             tc.tile_pool(name="psum", bufs=2, space="PSUM") as ppool, \
             tc.tile_pool(name="outp", bufs=2) as opool:
            wsb = wpool.tile([128, (W_TILES + 1) * 128], f32r)
            for t in range(W_TILES + 1):
                nc.sync.dma_start(wsb[:, 128 * t: 128 * (t + 1)], wb[t].bitcast(f32r))
            for g in range(GROUPS):
                data = dpool.tile([128, W_TILES * F], f32r)
                for t in range(W_TILES):
                    nc.sync.dma_start(
                        data[:, F * t: F * (t + 1)],
                        kx[GROUP_ROWS * g + 128 * t: GROUP_ROWS * g + 128 * (t + 1), :]
                        .bitcast(f32r))
                tail = tpool.tile([KERNEL_STRIDE, F], f32r)
                nc.sync.dma_start(
                    tail[:],
                    kx[GROUP_ROWS * g + 2048: GROUP_ROWS * g + 2048 + KERNEL_STRIDE, :]
                    .bitcast(f32r))

                ps = ppool.tile([128, F], mybir.dt.float32)
                for t in range(W_TILES):
                    nc.tensor.matmul(
                        ps[:],
                        lhsT=wsb[:, 128 * t: 128 * (t + 1)],
                        rhs=data[:, F * t: F * (t + 1)],
                        start=(t == 0), stop=False)
                nc.tensor.matmul(
                    ps[:],
                    lhsT=wsb[0:KERNEL_STRIDE, 128 * W_TILES: 128 * (W_TILES + 1)],
                    rhs=tail[:],
                    start=False, stop=True)

                ot = opool.tile([128, F], mybir.dt.float32)
                nc.scalar.copy(ot[:], ps[:])
                nc.scalar.dma_start(out[128 * g: 128 * (g + 1), :], ot[:])
    nc.compile()
    return nc


def _get_module():
    if "nc" not in _CACHE:
        _CACHE["nc"] = _build_module()
    return _CACHE["nc"]


def _calc_chunks_with_stride(cu_seqlens_np, chunk_size, stride):
    """Host-side mirror of the reference index computation."""
    cu = np.asarray(cu_seqlens_np, dtype=np.int64)
    batch_sizes = cu[1:] - cu[:-1]
    max_seq_len = int(batch_sizes.max())
    max_chunks = max((max_seq_len - chunk_size) // stride + 1, 0)
    offsets = np.arange(0, max_chunks * stride, stride, dtype=np.int64)
    seq_starts = cu[:-1]
    chunk_start = seq_starts[:, None] + offsets[None, :]
    chunk_end = chunk_start + chunk_size
    valid = chunk_end <= (seq_starts[:, None] + batch_sizes[:, None])
    valid_starts = chunk_start[valid]
    inner = np.arange(chunk_size, dtype=np.int64)[None, :]
    flat_idx = (valid_starts[:, None] + inner).reshape(-1)
    n_per_batch = valid.sum(axis=1)
    cu_comp = np.zeros(len(cu), dtype=np.int32)
    cu_comp[1:] = np.cumsum(n_per_batch)
    return flat_idx, cu_comp


def _numpy_fallback(k, cu_seqlens):
    flat_idx, cu_comp = _calc_chunks_with_stride(
        np.asarray(cu_seqlens), KERNEL_SIZE, KERNEL_STRIDE)
    k = np.asarray(k)
    gathered = k[flat_idx].reshape(-1, KERNEL_SIZE, k.shape[1], k.shape[2])
    return gathered.mean(axis=1, dtype=np.float64).astype(k.dtype), cu_comp


def _run_hw(k2: np.ndarray, trace: bool = False, **spmd_kwargs):
    """k2: (BATCH*SEQ_LEN, F) fp32 contiguous. Returns (per-core outs, results obj)."""
    from concourse.bass_utils import run_bass_kernel_spmd

    nc = _get_module()
    wb = _CACHE.setdefault("wb", _build_weights())
    in_maps = []
    for s in range(BATCH):
        base = s * SEQ_LEN
        in_maps.append({"kx": k2[base: base + ROWS_PER_CORE], "wb": wb})
        in_maps.append({"kx": k2[base + SEQ_LEN - ROWS_PER_CORE: base + SEQ_LEN],
                        "wb": wb})
    res = run_bass_kernel_spmd(nc, in_maps, core_ids=list(range(N_CORES)),
                               trace=trace, **spmd_kwargs)
    outs = [res.results[i]["out"] for i in range(N_CORES)]
    return outs, res


def _assemble(outs) -> np.ndarray:
    seqs = []
    for s in range(BATCH):
        a = outs[2 * s]          # chunks 0..511
        b = outs[2 * s + 1]      # chunks 511..1022 (first is dup of a[511])
        seqs.append(np.concatenate([a, b[1:]], axis=0))
    comp = np.concatenate(seqs, axis=0)
    return np.ascontiguousarray(comp.reshape(-1, HEAD_NUM_K, HEAD_DIM))


def kernel(k, cu_seqlens):
    k = np.asarray(k)
    cu_seqlens = np.asarray(cu_seqlens)
    expected_cu = np.arange(BATCH + 1, dtype=np.int64) * SEQ_LEN
    if (k.shape != (BATCH * SEQ_LEN, HEAD_NUM_K, HEAD_DIM)
            or k.dtype != np.float32
            or cu_seqlens.shape != (BATCH + 1,)
            or not np.array_equal(np.asarray(cu_seqlens, np.int64), expected_cu)):
        return _numpy_fallback(k, cu_seqlens)

    _, cu_comp = _calc_chunks_with_stride(cu_seqlens, KERNEL_SIZE, KERNEL_STRIDE)
    k2 = np.ascontiguousarray(k.reshape(BATCH * SEQ_LEN, F))
    outs, _ = _run_hw(k2)
    return _assemble(outs), cu_comp
